# revision 40
# baseline (speedup 1.0000x reference)
"""Causal self-attention (B=2, T=2048, C=1024, H=16, Dh=64) on 8 TRN2 cores.

Sharding: data-parallel over B (2) x tensor-parallel over heads (4 groups of
4 heads) = 8 shards. Core i handles batch i//4, heads 4*(i%4)..4*(i%4)+3.
Host pre-marshals each shard's operands (slice + transpose to contraction-
major + cast to bf16, standard tensor-parallel weight layout); each core
computes its QKV projection, causal-softmax attention for its 4 heads, and
its partial out-projection. Host sums the 4 bf16 partials per batch
(row-parallel out-projection reduce) in f64.

Device program (per core, all matmuls bf16 with f32 PSUM accumulation):
  xt  [1024, 2048] bf16 = x[b].T
  wt  [1024, 768]  bf16 = Wqkv_shard.T   (f = Qp0|Qp1|Kp0|Kp1|V)
  wot [256, 1024]  bf16 = Wout[:, cols].T
  y   [2048, 1024] bf16 partial output

  1. qkT[f, t] = sum_c wt[c, f] xt[c, t]      (Q^T, K^T head-pair tiles)
  2. v[t, f]   = sum_c xt[c, t] wt[c, 512+f]  (V tiles + ones column)
  3. per head pair (ST halves ride concurrent PE row-tiles h0/h64):
       ST[k, q] = exp(0.125 * sum_d K^T[d, k] Q^T[d, q]) (causal-masked)
       outT[d', q] += V[k, d'] ST[k, q]   (d'=65: ones col accumulates Z)
       OUTT[c', q] = outT[c', q] * (1/Z[q])
  4. y[t, f] = sum_c' OUTT[c', t] wot[c', f]

Schedule: QKV projections + out-projection ride as fine-grained PE filler
inside the ACT(exp)-paced attention k-loops; the two head pairs' blocks are
interleaved (0,0)(0,1)(0,2)(1,1)(1,2)(1,3)(0,3)(1,0) so late work spreads
across all windows. Each block's softmax normalization is DMA-free: ACT
copies the raw Z row to SBUF, a PE outer-product broadcasts it into the
score-PSUM rotation, and a 64-lane approximate reciprocal + two DVE
multiplies produce the normalized OUTT; the heavy half is deferred into the
next block's k-loop so no engine ever stalls on it. Dummy warm-up matmuls
on the constant mask lift the PE HAM clock gate to 2.4 GHz before the real
data lands.
"""

import sys

for _p in ("/opt/trn_rl_repo",):
    if _p not in sys.path:
        sys.path.append(_p)

import numpy as np
import ml_dtypes
from contextlib import ExitStack

import concourse.bass as bass
import concourse.bacc as bacc
import concourse.mybir as mybir
import concourse.tile as tile
from concourse.bass_utils import run_bass_kernel_spmd
from concourse.masks import make_upper_triangular

BF16 = mybir.dt.bfloat16
F32 = mybir.dt.float32
AF = mybir.ActivationFunctionType

T = 2048
C = 1024
N_CORES = 8

_cached_nc = None


def build_program():
    global _cached_nc
    if _cached_nc is not None:
        return _cached_nc
    nc = bacc.Bacc("TRN2", target_bir_lowering=False, debug=False,
                   num_devices=N_CORES)
    xt_d = nc.dram_tensor("xt", [C, T], BF16, kind="ExternalInput").ap()
    wt_d = nc.dram_tensor("wt", [C, 768], BF16, kind="ExternalInput").ap()
    wot_d = nc.dram_tensor("wot", [256, C], BF16, kind="ExternalInput").ap()
    y_d = nc.dram_tensor("y", [T, C], BF16, kind="ExternalOutput").ap()

    with tile.TileContext(nc) as tc, ExitStack() as ctx:
        const = ctx.enter_context(tc.tile_pool(name="const", bufs=1))
        sb = ctx.enter_context(tc.tile_pool(name="sb", bufs=1))
        wk = ctx.enter_context(tc.tile_pool(name="wk", bufs=1))
        ps = ctx.enter_context(tc.tile_pool(name="ps", bufs=1, space="PSUM"))

        trimask = const.tile([128, 128], BF16, tag="trimask")
        make_upper_triangular(nc, trimask[:], val=1.0, diag=True)
        zbias = const.tile([128, 1], F32, tag="zbias")
        nc.vector.memset(zbias[:], 0.0)
        onesrow = const.tile([1, 64], BF16, tag="onesrow")
        nc.vector.memset(onesrow[:], 1.0)

        XTC = [sb.tile([128, 4096], BF16, tag=f"xtc{nb}", name=f"xtcs{nb}")
               for nb in range(4)]

        def XTs(kc, nb):
            return XTC[nb][:, kc * 512:(kc + 1) * 512]
        WT = [sb.tile([128, 768], BF16, tag=f"wt{k}", name=f"wts{k}")
              for k in range(8)]
        WOT = [sb.tile([128, C], BF16, tag=f"wot{k}", name=f"wots{k}")
               for k in range(2)]
        QT = [sb.tile([128, T], BF16, tag=f"qt{p}", name=f"qts{p}")
              for p in range(2)]
        KT = [sb.tile([128, T], BF16, tag=f"kt{p}", name=f"kts{p}")
              for p in range(2)]
        V = [sb.tile([128, 4 * 65], BF16, tag=f"v{t}", name=f"vs{t}")
             for t in range(16)]
        OUTT = [sb.tile([128, T], BF16, tag=f"outt{p}", name=f"outts{p}")
                for p in range(2)]

        # DMA plan: weights on the scalar HWDGE queue (issues done by ~12us,
        # scalar engine free before the exp chain starts); ALL xt column
        # blocks + y on the sync queue; wot on the gpsimd SWDGE queue
        # (needed only by the out-projection, ~60us in).
        for k in range(8):
            nc.scalar.dma_start(WT[k][:], wt_d[128 * k:128 * (k + 1), :])
        for nb in range(4):
            for k in range(8):
                nc.sync.dma_start(
                    XTs(k, nb),
                    xt_d[128 * k:128 * (k + 1), 512 * nb:512 * (nb + 1)])
        for k in range(2):
            nc.gpsimd.dma_start(WOT[k][:], wot_d[128 * k:128 * (k + 1), :])

        # PSUM budget (8 banks): "st" [128,1024]x2 = 4 (attention scores +
        # the Z outer-product broadcast riding the rotation), "pv"
        # [128,1024]x1 = 2 (attention out), "pj" [128,512]x2 = 2
        # (projections + out-projection).

        # HAM warm-up: ~21 dummy matmuls on the constant mask (ready at
        # ~7us, no data dependencies) keep the PE busy through the clock
        # gate's activity window so the real prologue runs at 2.4 GHz.
        scratch = wk.tile([128, 128], F32, tag="scratch", name="scratch")
        for g in range(3):
            dps = ps.tile([128, 1024], F32, tag="st", bufs=2, name="stp")
            for i in range(7):
                nc.tensor.matmul(dps[:, 0:128], trimask[:], trimask[:],
                                 start=(i == 0), stop=(i == 6))
            nc.vector.tensor_copy(scratch[:], dps[:, 0:128])

        class Filler:
            """Queue of matmul-emission steps; fill(n) emits up to n matmuls
            of PE filler work (psum accumulation groups + their evictions)
            inside ACT-paced attention windows."""

            def __init__(self):
                self.steps = []   # flat list of callables, each emits 1 MM
                self.pos = 0

            def add_group(self, mk_steps):
                self.steps.extend(mk_steps)
                return len(self.steps)   # marker: position after this group

            def fill(self, n):
                end = min(self.pos + n, len(self.steps))
                while self.pos < end:
                    self.steps[self.pos]()
                    self.pos += 1

            def fill_until(self, marker):
                self.fill(max(0, marker - self.pos))

            def drain(self):
                self.fill(len(self.steps))

        def proj_qk_steps(p, which, nb):
            dst = QT[p] if which == 0 else KT[p]
            fb = p * 128 + (0 if which == 0 else 256)
            box = {}

            def step(kc):
                def go():
                    if kc == 0:
                        box['pj'] = ps.tile([128, 512], F32, tag="pj",
                                            bufs=2, name="pj")
                    nc.tensor.matmul(
                        box['pj'][:],
                        WT[kc][:, fb:fb + 128],
                        XTs(kc, nb),
                        start=(kc == 0), stop=(kc == 7))
                    if kc == 7:
                        nc.vector.tensor_copy(
                            dst[:, nb * 512:(nb + 1) * 512], box['pj'][:])
                return go
            return [step(kc) for kc in range(8)]

        def proj_v_steps(tt):
            box = {}

            def step(kc):
                def go():
                    if kc == 0:
                        box['pj'] = ps.tile([128, 512], F32, tag="pj",
                                            bufs=2, name="pj")
                    nc.tensor.matmul(
                        box['pj'][:, 0:256],
                        XTs(kc, tt // 4)[:, (tt % 4) * 128:(tt % 4 + 1) * 128],
                        WT[kc][:, 512:768],
                        start=(kc == 0), stop=(kc == 7))
                    if kc == 7:
                        pj = box['pj']
                        vv = V[tt].rearrange("p (h e) -> p h e", e=65)
                        nc.vector.tensor_copy(
                            vv[:, :, 0:64],
                            pj[:, 0:256].rearrange("p (h e) -> p h e", e=64))
                        nc.vector.memset(vv[:, :, 64:65], 1.0)
                return go
            return [step(kc) for kc in range(8)]

        def outproj_steps(tt, evict_split=False):
            """4 matmuls producing y[tt*128:(tt+1)*128, :]. evict_split:
            route the two psum evictions to DVE and ACT (post-exp tail,
            both engines free) instead of DVE only."""
            box = {}

            def step(fb, kcp):
                def go():
                    if fb == 0 and kcp == 0:
                        box['ysb'] = wk.tile([128, C], BF16, tag="ysb",
                                             bufs=2, name="ysb")
                    if kcp == 0:
                        box['pj'] = ps.tile([128, 512], F32, tag="pj",
                                            bufs=2, name="pj")
                    nc.tensor.matmul(
                        box['pj'][:],
                        OUTT[kcp][:, tt * 128:(tt + 1) * 128],
                        WOT[kcp][:, fb * 512:(fb + 1) * 512],
                        start=(kcp == 0), stop=(kcp == 1))
                    if kcp == 1:
                        if evict_split and fb == 1:
                            nc.scalar.copy(
                                box['ysb'][:, fb * 512:(fb + 1) * 512],
                                box['pj'][:])
                        else:
                            nc.vector.tensor_copy(
                                box['ysb'][:, fb * 512:(fb + 1) * 512],
                                box['pj'][:])
                        if evict_split:
                            # tail groups: ship each half immediately so the
                            # final transfer before teardown is small
                            nc.sync.dma_start(
                                y_d[tt * 128:(tt + 1) * 128,
                                    fb * 512:(fb + 1) * 512],
                                box['ysb'][:, fb * 512:(fb + 1) * 512])
                    if fb == 1 and kcp == 1 and not evict_split:
                        nc.sync.dma_start(y_d[tt * 128:(tt + 1) * 128, :],
                                          box['ysb'][:])
                return go
            return [step(fb, kcp) for fb in range(2) for kcp in range(2)]

        # Deferred normalize: after a block's last PV, only the cheap
        # phase-1 evictions are emitted (DVE copy of the unnormalized out,
        # ACT copy of the raw Z row to bf16 SBUF). The heavy phase 2 (PE
        # outer-product broadcast into the st rotation, 64-lane approx
        # reciprocal, the two OUTT multiplies) fires inside the NEXT
        # block's k-loop, when its inputs are long ready.
        pending = {'norm': None, 'after': None}

        def normalize_phase1(p, qb, pv, sliced=False):
            qsl = slice(qb * 512, (qb + 1) * 512)
            u = wk.tile([64, 1024], F32, tag="u", bufs=2, name="u")
            nc.vector.tensor_copy(u[:], pv[0:64, :])
            zrawb = wk.tile([1, 1024], BF16, tag="zrawb", bufs=2,
                            name="zrawb")
            with nc.allow_low_precision(
                    reason="Z row in bf16; |rel err| ~0.4% matches the bf16 "
                           "OUTT quantization already present"):
                nc.scalar.copy(zrawb[:], pv[64:65, :])

            def phase2():
                # broadcast the raw Z row via a bf16 PE outer-product into
                # the st rotation, then 64-lane approx reciprocal; no DMA
                # on this chain.
                zst = ps.tile([128, 1024], F32, tag="st", bufs=2, name="stp")
                nc.tensor.matmul(zst[0:64, 0:512],
                                 onesrow[:],
                                 zrawb[:, 0:512], start=True, stop=True)
                nc.tensor.matmul(zst[0:64, 512:1024],
                                 onesrow[:],
                                 zrawb[:, 512:1024], start=True, stop=True)
                zb = wk.tile([64, 1024], F32, tag="zb", bufs=2, name="zb")
                nc.vector.reciprocal_approx_fast(zb[:], zst[0:64, :])
                if sliced:
                    # per-128-col slices so trailing out-projection groups
                    # unblock as soon as their slice is normalized
                    for t in range(4):
                        tsl = slice(qb * 512 + t * 128,
                                    qb * 512 + t * 128 + 128)
                        ua = slice(t * 128, t * 128 + 128)
                        ub = slice(512 + t * 128, 512 + t * 128 + 128)
                        nc.vector.tensor_mul(OUTT[p][0:64, tsl], u[:, ua],
                                             zb[:, ua])
                        nc.vector.tensor_mul(OUTT[p][64:128, tsl], u[:, ub],
                                             zb[:, ub])
                else:
                    nc.vector.tensor_mul(OUTT[p][0:64, qsl], u[:, 0:512],
                                         zb[:, 0:512])
                    nc.vector.tensor_mul(OUTT[p][64:128, qsl], u[:, 512:1024],
                                         zb[:, 512:1024])
            return phase2

        def attention_qb(p, qb, filler, rate, final_cb=None,
                         sliced_norm=False):
            hA, hB = 2 * p, 2 * p + 1
            # merged A/B psum: head A in cols 0:512, head B in 512:1024
            pv = ps.tile([128, 1024], F32, tag="pv", bufs=1, name="pv")
            nkt = (qb + 1) * 4

            def emit_pv(kt, sa, off, ncols):
                nc.tensor.matmul(
                    pv[0:65, off:512],
                    V[kt][:, hA * 65:hA * 65 + 65],
                    sa[:, 0:ncols],
                    start=(kt == 0), stop=(kt == nkt - 1))
                nc.tensor.matmul(
                    pv[0:65, 512 + off:1024],
                    V[kt][:, hB * 65:hB * 65 + 65],
                    sa[:, 512:512 + ncols],
                    start=(kt == 0), stop=(kt == nkt - 1))

            # software-pipelined in kt PAIRS: the four 64-row ST matmuls of
            # a pair run back-to-back so each LDWEIGHTS overlaps the other
            # head's in-flight stream (disjoint row groups); PVs of the
            # previous pair are emitted after the current pair's STs so the
            # in-order PE never waits on an exp.
            pend_pv = []
            for kt0 in range(0, nkt, 2):
                pair = []
                for kt in (kt0, kt0 + 1):
                    off = max(0, kt * 128 - qb * 512)
                    ncols = 512 - off
                    qs = qb * 512 + off
                    stp = ps.tile([128, 1024], F32, tag="st", bufs=2,
                                  name="stp")
                    nc.tensor.matmul(
                        stp[:, 0:ncols],
                        KT[p][0:64, kt * 128:(kt + 1) * 128],
                        QT[p][0:64, qs:qs + ncols],
                        start=True, stop=True)
                    nc.tensor.matmul(
                        stp[:, 512:512 + ncols],
                        KT[p][64:128, kt * 128:(kt + 1) * 128],
                        QT[p][64:128, qs:qs + ncols],
                        start=True, stop=True)
                    pair.append((kt, stp, off, ncols))
                for kt, stp, off, ncols in pair:
                    sa = wk.tile([128, 1024], BF16, tag="sa_sb", bufs=4,
                                 name="sa")
                    nc.scalar.activation(
                        sa.rearrange("p (g n) -> p g n", g=2)[:, :, 0:ncols],
                        stp.rearrange("p (g n) -> p g n", g=2)[:, :, 0:ncols],
                        AF.Exp, bias=zbias[:], scale=0.125)
                    if off > 0 or kt * 128 == qb * 512:
                        m3 = sa.rearrange("p (g n) -> p g n", g=2)[:, :, 0:128]
                        nc.vector.tensor_mul(
                            m3, m3,
                            trimask[:].unsqueeze(1).broadcast_to([128, 2, 128]))
                    pend_pv.append((kt, sa, off, ncols))
                if kt0 == 2 and pending['norm'] is not None:
                    pending['norm']()
                    pending['norm'] = None
                    if pending['after'] is not None:
                        pending['after']()
                        pending['after'] = None
                if filler is not None:
                    filler.fill(2 * rate)
                while len(pend_pv) > 2:
                    emit_pv(*pend_pv.pop(0))
            for args in pend_pv:
                emit_pv(*args)

            if final_cb is not None:
                # Last attention block: inline normalize, per-128-col
                # slices feeding the final out-projection groups.
                qsl0 = qb * 512
                # both final evictions ride the (now idle) ACT engine, Z row
                # first, so the DVE can go straight from the previous
                # block's reciprocal to its normalize multiplies
                zrawb = wk.tile([1, 1024], BF16, tag="zrawb", bufs=2,
                                name="zrawb")
                with nc.allow_low_precision(
                        reason="Z row in bf16; |rel err| ~0.4% matches the "
                               "bf16 OUTT quantization already present"):
                    nc.scalar.copy(zrawb[:], pv[64:65, :])
                u = wk.tile([64, 1024], F32, tag="u", bufs=2, name="u")
                nc.scalar.copy(u[:], pv[0:64, :])
                if filler is not None:
                    filler.fill(6)   # cover the Z-copy latency only
                zst = ps.tile([128, 1024], F32, tag="st", bufs=2, name="stp")
                nc.tensor.matmul(zst[0:64, 0:512],
                                 onesrow[:],
                                 zrawb[:, 0:512], start=True, stop=True)
                nc.tensor.matmul(zst[0:64, 512:1024],
                                 onesrow[:],
                                 zrawb[:, 512:1024], start=True, stop=True)
                if filler is not None:
                    filler.drain()   # PE works while the reciprocal runs
                zb = wk.tile([64, 1024], F32, tag="zb", bufs=2, name="zb")
                nc.vector.reciprocal_approx_fast(zb[:], zst[0:64, :])
                for t in range(4):
                    tsl = slice(qsl0 + t * 128, qsl0 + t * 128 + 128)
                    ua = slice(t * 128, t * 128 + 128)
                    ub = slice(512 + t * 128, 512 + t * 128 + 128)
                    nc.vector.tensor_mul(OUTT[p][0:64, tsl], u[:, ua],
                                         zb[:, ua])
                    nc.vector.tensor_mul(OUTT[p][64:128, tsl], u[:, ub],
                                         zb[:, ub])
                    final_cb(qb * 4 + t)
                return None
            return normalize_phase1(p, qb, pv, sliced=sliced_norm)

        # Emission order = PE order (in-order engine). Minimal prefix:
        # dummy warm-ups, then Q0/K0 nb=0 interleaved per xt chunk (matches
        # the chunk arrival order) + V[0..3], unblocking attention (0,0)
        # right after the first 1 MB of xt lands. Everything else rides as
        # fine-grained PE filler inside the ACT-paced attention k-loops,
        # force-drained just before the attention block that consumes it.
        # chunk-major prologue: Q, K and all four V accumulations per xt
        # chunk (~1.5us of PE work per ~1.3us chunk arrival) so the PE never
        # idles while the remaining nb0 chunks land. The four V tiles share
        # one spare score-PSUM buffer (4 x 256 f32 columns).
        sQ = proj_qk_steps(0, 0, 0)
        sK = proj_qk_steps(0, 1, 0)
        vstA = ps.tile([128, 1024], F32, tag="st", bufs=2, name="stp")
        vstB = ps.tile([128, 1024], F32, tag="st", bufs=2, name="stp")
        # one V accumulator per PSUM bank (start-flag state is per bank)
        vslot = [(vstA, 0), (vstA, 512), (vstB, 0), (vstB, 512)]
        for kc in range(8):
            sQ[kc]()
            sK[kc]()
            for tt in range(4):
                vt, cb = vslot[tt]
                nc.tensor.matmul(
                    vt[:, cb:cb + 256],
                    XTs(kc, 0)[:, (tt % 4) * 128:(tt % 4 + 1) * 128],
                    WT[kc][:, 512:768],
                    start=(kc == 0), stop=(kc == 7))
        for tt in range(4):
            vt, cb = vslot[tt]
            vv = V[tt].rearrange("p (h e) -> p h e", e=65)
            nc.vector.tensor_copy(
                vv[:, :, 0:64],
                vt[:, cb:cb + 256].rearrange("p (h e) -> p h e", e=64))
            nc.vector.memset(vv[:, :, 64:65], 1.0)
        fill = Filler()
        marks = {}

        def dl(block_idx, groups):
            m = 0
            for g in groups:
                m = fill.add_group(g)
            marks[block_idx] = m

        dl(1, [proj_qk_steps(0, w, 1) for w in range(2)] +
              [proj_v_steps(tt) for tt in range(4, 8)])
        dl(2, [proj_qk_steps(0, w, 2) for w in range(2)] +
              [proj_v_steps(tt) for tt in range(8, 12)])
        dl(3, [proj_qk_steps(1, 0, 1), proj_qk_steps(1, 1, 0),
               proj_qk_steps(1, 1, 1)])
        dl(4, [proj_qk_steps(1, 0, 2), proj_qk_steps(1, 1, 2)])
        dl(5, [proj_qk_steps(1, 0, 3), proj_qk_steps(1, 1, 3)] +
              [proj_v_steps(tt) for tt in range(12, 16)])
        dl(6, [proj_qk_steps(0, w, 3) for w in range(2)])
        dl(7, [proj_qk_steps(1, 0, 0)])

        def add_outproj(tts, split=False):
            def go():
                for tt in tts:
                    fill.add_group(outproj_steps(tt, evict_split=split))
            return go

        def final_outproj(tt):
            for s in outproj_steps(tt, evict_split=True):
                s()

        SEQ = [(0, 0), (0, 1), (0, 2), (1, 1), (1, 2), (1, 3), (0, 3),
               (1, 0)]
        RATES = [4, 4, 4, 3, 2, 2, 2, 2]
        AFTER = {3: add_outproj(range(4, 8)), 4: add_outproj(range(8, 12)),
                 6: add_outproj(range(12, 16), split=True)}
        for bi, (p, qb) in enumerate(SEQ):
            if bi in marks:
                fill.fill_until(marks[bi])
            ph2 = attention_qb(p, qb, filler=fill, rate=RATES[bi],
                               final_cb=final_outproj if bi == 7 else None,
                               sliced_norm=(bi == 6))
            pending['norm'] = ph2
            pending['after'] = AFTER.get(bi)
        fill.drain()

    nc.compile()
    _cached_nc = nc
    return nc


def shard_inputs(x, Wqkv, Wout):
    """Full inputs -> 8 per-core input dicts (sliced/transposed/bf16-cast)."""
    bf = ml_dtypes.bfloat16
    in_maps = []
    for i in range(N_CORES):
        b, g = divmod(i, 4)
        r = slice(256 * g, 256 * (g + 1))
        w_my = np.concatenate(
            [Wqkv[0:1024][r], Wqkv[1024:2048][r], Wqkv[2048:3072][r]], axis=0)
        in_maps.append({
            "xt": np.ascontiguousarray(x[b].T).astype(bf),
            "wt": np.ascontiguousarray(w_my.T).astype(bf),
            "wot": np.ascontiguousarray(Wout[:, r].T).astype(bf),
        })
    return in_maps


def gather_output(results):
    """8 per-core partial y (bf16) -> full [2, T, C] f32 output."""
    y = np.zeros((2, T, C), dtype=np.float64)
    for i in range(N_CORES):
        y[i // 4] += np.asarray(results[i]["y"], dtype=np.float64)
    return y.astype(np.float32)


def kernel(x, Wqkv, Wout):
    x = np.asarray(x)
    Wqkv = np.asarray(Wqkv)
    Wout = np.asarray(Wout)
    nc = build_program()
    in_maps = shard_inputs(x, Wqkv, Wout)
    res = run_bass_kernel_spmd(nc, in_maps, core_ids=list(range(N_CORES)))
    return gather_output(res.results)


# revision 41
# speedup vs baseline: 1.0397x; 1.0397x over previous
"""Causal self-attention (B=2, T=2048, C=1024, H=16, Dh=64) on 8 TRN2 cores.

Sharding: data-parallel over B (2) x tensor-parallel over heads (4 groups of
4 heads) = 8 shards. Core i handles batch i//4, heads 4*(i%4)..4*(i%4)+3.
Host pre-marshals each shard's operands (slice + transpose to contraction-
major + cast to bf16, standard tensor-parallel weight layout); each core
computes its QKV projection, causal-softmax attention for its 4 heads, and
its partial out-projection. Host sums the 4 bf16 partials per batch
(row-parallel out-projection reduce) in f64.

Device program (per core, all matmuls bf16 with f32 PSUM accumulation):
  xt  [1024, 2048] bf16 = x[b].T
  wt  [1024, 768]  bf16 = Wqkv_shard.T   (f = Qp0|Qp1|Kp0|Kp1|V)
  wot [256, 1024]  bf16 = Wout[:, cols].T
  y   [2048, 1024] bf16 partial output

  1. qkT[f, t] = sum_c wt[c, f] xt[c, t]      (Q^T, K^T head-pair tiles)
  2. v[t, f]   = sum_c xt[c, t] wt[c, 512+f]  (V tiles + ones column)
  3. per head pair (ST halves ride concurrent PE row-tiles h0/h64):
       ST[k, q] = exp(0.125 * sum_d K^T[d, k] Q^T[d, q]) (causal-masked)
       outT[d', q] += V[k, d'] ST[k, q]   (d'=65: ones col accumulates Z)
       OUTT[c', q] = outT[c', q] * (1/Z[q])
  4. y[t, f] = sum_c' OUTT[c', t] wot[c', f]

Schedule: QKV projections + out-projection ride as fine-grained PE filler
inside the ACT(exp)-paced attention k-loops; the two head pairs' blocks are
interleaved (0,0)(0,1)(0,2)(1,1)(1,2)(1,3)(0,3)(1,0) so late work spreads
across all windows. Each block's softmax normalization is DMA-free: ACT
copies the raw Z row to SBUF, a PE outer-product broadcasts it into the
score-PSUM rotation, and a 64-lane approximate reciprocal + two DVE
multiplies produce the normalized OUTT; the heavy half is deferred into the
next block's k-loop so no engine ever stalls on it. Dummy warm-up matmuls
on the constant mask lift the PE HAM clock gate to 2.4 GHz before the real
data lands.
"""

import sys

for _p in ("/opt/trn_rl_repo",):
    if _p not in sys.path:
        sys.path.append(_p)

import numpy as np
import ml_dtypes
from contextlib import ExitStack

import concourse.bass as bass
import concourse.bacc as bacc
import concourse.mybir as mybir
import concourse.tile as tile
from concourse.bass_utils import run_bass_kernel_spmd
from concourse.masks import make_upper_triangular

BF16 = mybir.dt.bfloat16
F32 = mybir.dt.float32
AF = mybir.ActivationFunctionType

T = 2048
C = 1024
N_CORES = 8

_cached_nc = None


def build_program():
    global _cached_nc
    if _cached_nc is not None:
        return _cached_nc
    nc = bacc.Bacc("TRN2", target_bir_lowering=False, debug=False,
                   num_devices=N_CORES)
    xt_d = nc.dram_tensor("xt", [C, T], BF16, kind="ExternalInput").ap()
    wt_d = nc.dram_tensor("wt", [C, 768], BF16, kind="ExternalInput").ap()
    wot_d = nc.dram_tensor("wot", [256, C], BF16, kind="ExternalInput").ap()
    y_d = nc.dram_tensor("y", [T, C], BF16, kind="ExternalOutput").ap()

    with tile.TileContext(nc) as tc, ExitStack() as ctx:
        const = ctx.enter_context(tc.tile_pool(name="const", bufs=1))
        sb = ctx.enter_context(tc.tile_pool(name="sb", bufs=1))
        wk = ctx.enter_context(tc.tile_pool(name="wk", bufs=1))
        ps = ctx.enter_context(tc.tile_pool(name="ps", bufs=1, space="PSUM"))

        trimask = const.tile([128, 128], BF16, tag="trimask")
        make_upper_triangular(nc, trimask[:], val=1.0, diag=True)
        zbias = const.tile([128, 1], F32, tag="zbias")
        nc.vector.memset(zbias[:], 0.0)
        onesrow = const.tile([1, 64], BF16, tag="onesrow")
        nc.vector.memset(onesrow[:], 1.0)

        XTC = [sb.tile([128, 4096], BF16, tag=f"xtc{nb}", name=f"xtcs{nb}")
               for nb in range(4)]

        def XTs(kc, nb):
            return XTC[nb][:, kc * 512:(kc + 1) * 512]
        WT = [sb.tile([128, 768], BF16, tag=f"wt{k}", name=f"wts{k}")
              for k in range(8)]
        WOT = [sb.tile([128, C], BF16, tag=f"wot{k}", name=f"wots{k}")
               for k in range(2)]
        QT = [sb.tile([128, T], BF16, tag=f"qt{p}", name=f"qts{p}")
              for p in range(2)]
        KT = [sb.tile([128, T], BF16, tag=f"kt{p}", name=f"kts{p}")
              for p in range(2)]
        V = [sb.tile([128, 4 * 65], BF16, tag=f"v{t}", name=f"vs{t}")
             for t in range(16)]
        OUTT = [sb.tile([128, T], BF16, tag=f"outt{p}", name=f"outts{p}")
                for p in range(2)]

        # DMA plan: weights on the scalar HWDGE queue (issues done by ~12us,
        # scalar engine free before the exp chain starts); ALL xt column
        # blocks + y on the sync queue; wot on the gpsimd SWDGE queue
        # (needed only by the out-projection, ~60us in).
        for k in range(8):
            nc.scalar.dma_start(WT[k][:], wt_d[128 * k:128 * (k + 1), :])
        for nb in range(4):
            for k in range(8):
                nc.sync.dma_start(
                    XTs(k, nb),
                    xt_d[128 * k:128 * (k + 1), 512 * nb:512 * (nb + 1)])
        for k in range(2):
            nc.gpsimd.dma_start(WOT[k][:], wot_d[128 * k:128 * (k + 1), :])

        # PSUM budget (8 banks): "st" [128,1024]x2 = 4 (attention scores +
        # the Z outer-product broadcast riding the rotation), "pv"
        # [128,1024]x1 = 2 (attention out), "pj" [128,512]x2 = 2
        # (projections + out-projection).

        # HAM warm-up: ~21 dummy matmuls on the constant mask (ready at
        # ~7us, no data dependencies) keep the PE busy through the clock
        # gate's activity window so the real prologue runs at 2.4 GHz.
        scratch = wk.tile([128, 128], F32, tag="scratch", name="scratch")
        for g in range(3):
            dps = ps.tile([128, 1024], F32, tag="st", bufs=2, name="stp")
            for i in range(7):
                nc.tensor.matmul(dps[:, 0:128], trimask[:], trimask[:],
                                 start=(i == 0), stop=(i == 6))
            nc.vector.tensor_copy(scratch[:], dps[:, 0:128])

        class Filler:
            """Queue of matmul-emission steps; fill(n) emits up to n matmuls
            of PE filler work (psum accumulation groups + their evictions)
            inside ACT-paced attention windows."""

            def __init__(self):
                self.steps = []   # flat list of callables, each emits 1 MM
                self.pos = 0

            def add_group(self, mk_steps):
                self.steps.extend(mk_steps)
                return len(self.steps)   # marker: position after this group

            def fill(self, n):
                end = min(self.pos + n, len(self.steps))
                while self.pos < end:
                    self.steps[self.pos]()
                    self.pos += 1

            def fill_until(self, marker):
                self.fill(max(0, marker - self.pos))

            def drain(self):
                self.fill(len(self.steps))

        def proj_qk_steps(p, which, nb):
            dst = QT[p] if which == 0 else KT[p]
            fb = p * 128 + (0 if which == 0 else 256)
            box = {}

            def step(kc):
                def go():
                    if kc == 0:
                        box['pj'] = ps.tile([128, 512], F32, tag="pj",
                                            bufs=2, name="pj")
                    nc.tensor.matmul(
                        box['pj'][:],
                        WT[kc][:, fb:fb + 128],
                        XTs(kc, nb),
                        start=(kc == 0), stop=(kc == 7))
                    if kc == 7:
                        nc.vector.tensor_copy(
                            dst[:, nb * 512:(nb + 1) * 512], box['pj'][:])
                return go
            return [step(kc) for kc in range(8)]

        def proj_v_steps(tt):
            box = {}

            def step(kc):
                def go():
                    if kc == 0:
                        box['pj'] = ps.tile([128, 512], F32, tag="pj",
                                            bufs=2, name="pj")
                    nc.tensor.matmul(
                        box['pj'][:, 0:256],
                        XTs(kc, tt // 4)[:, (tt % 4) * 128:(tt % 4 + 1) * 128],
                        WT[kc][:, 512:768],
                        start=(kc == 0), stop=(kc == 7))
                    if kc == 7:
                        pj = box['pj']
                        vv = V[tt].rearrange("p (h e) -> p h e", e=65)
                        nc.vector.tensor_copy(
                            vv[:, :, 0:64],
                            pj[:, 0:256].rearrange("p (h e) -> p h e", e=64))
                        nc.vector.memset(vv[:, :, 64:65], 1.0)
                return go
            return [step(kc) for kc in range(8)]

        def outproj_steps(tt, evict_split=False):
            """4 matmuls producing y[tt*128:(tt+1)*128, :]. evict_split:
            route the two psum evictions to DVE and ACT (post-exp tail,
            both engines free) instead of DVE only."""
            box = {}

            def step(fb, kcp):
                def go():
                    if fb == 0 and kcp == 0:
                        box['ysb'] = wk.tile([128, C], BF16, tag="ysb",
                                             bufs=2, name="ysb")
                    if kcp == 0:
                        box['pj'] = ps.tile([128, 512], F32, tag="pj",
                                            bufs=2, name="pj")
                    nc.tensor.matmul(
                        box['pj'][:],
                        OUTT[kcp][:, tt * 128:(tt + 1) * 128],
                        WOT[kcp][:, fb * 512:(fb + 1) * 512],
                        start=(kcp == 0), stop=(kcp == 1))
                    if kcp == 1:
                        if evict_split and fb == 1:
                            nc.scalar.copy(
                                box['ysb'][:, fb * 512:(fb + 1) * 512],
                                box['pj'][:])
                        else:
                            nc.vector.tensor_copy(
                                box['ysb'][:, fb * 512:(fb + 1) * 512],
                                box['pj'][:])
                        if evict_split:
                            # tail groups: ship each half immediately so the
                            # final transfer before teardown is small
                            nc.sync.dma_start(
                                y_d[tt * 128:(tt + 1) * 128,
                                    fb * 512:(fb + 1) * 512],
                                box['ysb'][:, fb * 512:(fb + 1) * 512])
                    if fb == 1 and kcp == 1 and not evict_split:
                        nc.sync.dma_start(y_d[tt * 128:(tt + 1) * 128, :],
                                          box['ysb'][:])
                return go
            return [step(fb, kcp) for fb in range(2) for kcp in range(2)]

        # Deferred normalize: after a block's last PV, only the cheap
        # phase-1 evictions are emitted (DVE copy of the unnormalized out,
        # ACT copy of the raw Z row to bf16 SBUF). The heavy phase 2 (PE
        # outer-product broadcast into the st rotation, 64-lane approx
        # reciprocal, the two OUTT multiplies) fires inside the NEXT
        # block's k-loop, when its inputs are long ready.
        pending = {'norm': None, 'after': None}

        def normalize_phase1(p, qb, pv, sliced=False):
            qsl = slice(qb * 512, (qb + 1) * 512)
            u = wk.tile([64, 1024], F32, tag="u", bufs=2, name="u")
            nc.vector.tensor_copy(u[:], pv[0:64, :])
            zrawb = wk.tile([1, 1024], BF16, tag="zrawb", bufs=2,
                            name="zrawb")
            with nc.allow_low_precision(
                    reason="Z row in bf16; |rel err| ~0.4% matches the bf16 "
                           "OUTT quantization already present"):
                nc.scalar.copy(zrawb[:], pv[64:65, :])

            def phase2():
                # broadcast the raw Z row via a bf16 PE outer-product into
                # the st rotation, then 64-lane approx reciprocal; no DMA
                # on this chain.
                zst = ps.tile([128, 1024], F32, tag="st", bufs=2, name="stp")
                nc.tensor.matmul(zst[0:64, 0:512],
                                 onesrow[:],
                                 zrawb[:, 0:512], start=True, stop=True)
                nc.tensor.matmul(zst[0:64, 512:1024],
                                 onesrow[:],
                                 zrawb[:, 512:1024], start=True, stop=True)
                zb = wk.tile([64, 1024], F32, tag="zb", bufs=2, name="zb")
                nc.vector.reciprocal_approx_fast(zb[:], zst[0:64, :])
                if sliced:
                    # per-128-col slices so trailing out-projection groups
                    # unblock as soon as their slice is normalized
                    for t in range(4):
                        tsl = slice(qb * 512 + t * 128,
                                    qb * 512 + t * 128 + 128)
                        ua = slice(t * 128, t * 128 + 128)
                        ub = slice(512 + t * 128, 512 + t * 128 + 128)
                        nc.vector.tensor_mul(OUTT[p][0:64, tsl], u[:, ua],
                                             zb[:, ua])
                        nc.vector.tensor_mul(OUTT[p][64:128, tsl], u[:, ub],
                                             zb[:, ub])
                else:
                    nc.vector.tensor_mul(OUTT[p][0:64, qsl], u[:, 0:512],
                                         zb[:, 0:512])
                    nc.vector.tensor_mul(OUTT[p][64:128, qsl], u[:, 512:1024],
                                         zb[:, 512:1024])
            return phase2

        def attention_qb(p, qb, filler, rate, final_cb=None,
                         sliced_norm=False):
            hA, hB = 2 * p, 2 * p + 1
            # merged A/B psum: head A in cols 0:512, head B in 512:1024
            pv = ps.tile([128, 1024], F32, tag="pv", bufs=1, name="pv")
            nkt = (qb + 1) * 4

            def emit_pv(kt, sa, off, ncols):
                nc.tensor.matmul(
                    pv[0:65, off:512],
                    V[kt][:, hA * 65:hA * 65 + 65],
                    sa[:, 0:ncols],
                    start=(kt == 0), stop=(kt == nkt - 1))
                nc.tensor.matmul(
                    pv[0:65, 512 + off:1024],
                    V[kt][:, hB * 65:hB * 65 + 65],
                    sa[:, 512:512 + ncols],
                    start=(kt == 0), stop=(kt == nkt - 1))

            # software-pipelined in kt PAIRS: the four 64-row ST matmuls of
            # a pair run back-to-back so each LDWEIGHTS overlaps the other
            # head's in-flight stream (disjoint row groups); PVs of the
            # previous pair are emitted after the current pair's STs so the
            # in-order PE never waits on an exp.
            pend_pv = []
            for kt0 in range(0, nkt, 2):
                pair = []
                for kt in (kt0, kt0 + 1):
                    off = max(0, kt * 128 - qb * 512)
                    ncols = 512 - off
                    qs = qb * 512 + off
                    stp = ps.tile([128, 1024], F32, tag="st", bufs=2,
                                  name="stp")
                    nc.tensor.matmul(
                        stp[:, 0:ncols],
                        KT[p][0:64, kt * 128:(kt + 1) * 128],
                        QT[p][0:64, qs:qs + ncols],
                        start=True, stop=True)
                    nc.tensor.matmul(
                        stp[:, 512:512 + ncols],
                        KT[p][64:128, kt * 128:(kt + 1) * 128],
                        QT[p][64:128, qs:qs + ncols],
                        start=True, stop=True)
                    pair.append((kt, stp, off, ncols))
                for kt, stp, off, ncols in pair:
                    sa = wk.tile([128, 1024], BF16, tag="sa_sb", bufs=4,
                                 name="sa")
                    nc.scalar.activation(
                        sa.rearrange("p (g n) -> p g n", g=2)[:, :, 0:ncols],
                        stp.rearrange("p (g n) -> p g n", g=2)[:, :, 0:ncols],
                        AF.Exp, bias=zbias[:], scale=0.125)
                    if off > 0 or kt * 128 == qb * 512:
                        m3 = sa.rearrange("p (g n) -> p g n", g=2)[:, :, 0:128]
                        nc.vector.tensor_mul(
                            m3, m3,
                            trimask[:].unsqueeze(1).broadcast_to([128, 2, 128]))
                    pend_pv.append((kt, sa, off, ncols))
                if kt0 == 2 and pending['norm'] is not None:
                    pending['norm']()
                    pending['norm'] = None
                    if pending['after'] is not None:
                        pending['after']()
                        pending['after'] = None
                if filler is not None:
                    filler.fill(2 * rate)
                while len(pend_pv) > 2:
                    emit_pv(*pend_pv.pop(0))
            for args in pend_pv:
                emit_pv(*args)

            if final_cb is not None:
                # Last attention block: inline normalize, per-128-col
                # slices feeding the final out-projection groups.
                qsl0 = qb * 512
                # both final evictions ride the (now idle) ACT engine, Z row
                # first, so the DVE can go straight from the previous
                # block's reciprocal to its normalize multiplies
                zrawb = wk.tile([1, 1024], BF16, tag="zrawb", bufs=2,
                                name="zrawb")
                with nc.allow_low_precision(
                        reason="Z row in bf16; |rel err| ~0.4% matches the "
                               "bf16 OUTT quantization already present"):
                    nc.scalar.copy(zrawb[:], pv[64:65, :])
                u = wk.tile([64, 1024], F32, tag="u", bufs=2, name="u")
                nc.scalar.copy(u[:], pv[0:64, :])
                if filler is not None:
                    filler.fill(6)   # cover the Z-copy latency only
                zst = ps.tile([128, 1024], F32, tag="st", bufs=2, name="stp")
                nc.tensor.matmul(zst[0:64, 0:512],
                                 onesrow[:],
                                 zrawb[:, 0:512], start=True, stop=True)
                nc.tensor.matmul(zst[0:64, 512:1024],
                                 onesrow[:],
                                 zrawb[:, 512:1024], start=True, stop=True)
                if filler is not None:
                    filler.drain()   # PE works while the reciprocal runs
                zb = wk.tile([64, 1024], F32, tag="zb", bufs=2, name="zb")
                nc.vector.reciprocal_approx_fast(zb[:], zst[0:64, :])
                for t in range(4):
                    tsl = slice(qsl0 + t * 128, qsl0 + t * 128 + 128)
                    ua = slice(t * 128, t * 128 + 128)
                    ub = slice(512 + t * 128, 512 + t * 128 + 128)
                    nc.vector.tensor_mul(OUTT[p][0:64, tsl], u[:, ua],
                                         zb[:, ua])
                    nc.vector.tensor_mul(OUTT[p][64:128, tsl], u[:, ub],
                                         zb[:, ub])
                    final_cb(qb * 4 + t)
                return None
            return normalize_phase1(p, qb, pv, sliced=sliced_norm)

        # Emission order = PE order (in-order engine). Minimal prefix:
        # dummy warm-ups, then Q0/K0 nb=0 interleaved per xt chunk (matches
        # the chunk arrival order) + V[0..3], unblocking attention (0,0)
        # right after the first 1 MB of xt lands. Everything else rides as
        # fine-grained PE filler inside the ACT-paced attention k-loops,
        # force-drained just before the attention block that consumes it.
        # chunk-major prologue: Q, K and all four V accumulations per xt
        # chunk (~1.5us of PE work per ~1.3us chunk arrival) so the PE never
        # idles while the remaining nb0 chunks land. The four V tiles share
        # one spare score-PSUM buffer (4 x 256 f32 columns).
        sQ = proj_qk_steps(0, 0, 0)
        sK = proj_qk_steps(0, 1, 0)
        vstA = ps.tile([128, 1024], F32, tag="st", bufs=2, name="stp")
        vstB = ps.tile([128, 1024], F32, tag="st", bufs=2, name="stp")
        # one V accumulator per PSUM bank (start-flag state is per bank)
        vslot = [(vstA, 0), (vstA, 512), (vstB, 0), (vstB, 512)]
        for kc in range(8):
            sQ[kc]()
            sK[kc]()
            for tt in range(4):
                vt, cb = vslot[tt]
                nc.tensor.matmul(
                    vt[:, cb:cb + 256],
                    XTs(kc, 0)[:, (tt % 4) * 128:(tt % 4 + 1) * 128],
                    WT[kc][:, 512:768],
                    start=(kc == 0), stop=(kc == 7))
        for tt in range(4):
            vt, cb = vslot[tt]
            vv = V[tt].rearrange("p (h e) -> p h e", e=65)
            nc.vector.tensor_copy(
                vv[:, :, 0:64],
                vt[:, cb:cb + 256].rearrange("p (h e) -> p h e", e=64))
            nc.vector.memset(vv[:, :, 64:65], 1.0)
        fill = Filler()
        marks = {}

        def dl(block_idx, groups):
            m = 0
            for g in groups:
                m = fill.add_group(g)
            marks[block_idx] = m

        dl(1, [proj_qk_steps(0, w, 1) for w in range(2)] +
              [proj_v_steps(tt) for tt in range(4, 8)])
        dl(2, [proj_qk_steps(0, w, 2) for w in range(2)] +
              [proj_v_steps(tt) for tt in range(8, 12)])
        dl(3, [proj_qk_steps(1, 0, 1), proj_qk_steps(1, 1, 0),
               proj_qk_steps(1, 1, 1)])
        dl(4, [proj_qk_steps(1, 0, 2), proj_qk_steps(1, 1, 2)])
        dl(5, [proj_qk_steps(1, 0, 3), proj_qk_steps(1, 1, 3)] +
              [proj_v_steps(tt) for tt in range(12, 16)])
        dl(6, [proj_qk_steps(0, w, 3) for w in range(2)])
        dl(7, [proj_qk_steps(1, 0, 0)])

        def add_outproj(tts, split=False):
            def go():
                for tt in tts:
                    fill.add_group(outproj_steps(tt, evict_split=split))
            return go

        def final_outproj(tt):
            for s in outproj_steps(tt, evict_split=True):
                s()

        SEQ = [(0, 0), (0, 1), (0, 2), (1, 1), (1, 2), (1, 3), (0, 3),
               (1, 0)]
        RATES = [4, 4, 4, 3, 2, 2, 2, 2]
        AFTER = {3: add_outproj(range(4, 8)), 4: add_outproj(range(8, 12)),
                 6: add_outproj(range(12, 16))}
        for bi, (p, qb) in enumerate(SEQ):
            if bi in marks:
                fill.fill_until(marks[bi])
            ph2 = attention_qb(p, qb, filler=fill, rate=RATES[bi],
                               final_cb=final_outproj if bi == 7 else None,
                               sliced_norm=(bi == 6))
            pending['norm'] = ph2
            pending['after'] = AFTER.get(bi)
        fill.drain()

    nc.compile()
    _cached_nc = nc
    return nc


def shard_inputs(x, Wqkv, Wout):
    """Full inputs -> 8 per-core input dicts (sliced/transposed/bf16-cast)."""
    bf = ml_dtypes.bfloat16
    in_maps = []
    for i in range(N_CORES):
        b, g = divmod(i, 4)
        r = slice(256 * g, 256 * (g + 1))
        w_my = np.concatenate(
            [Wqkv[0:1024][r], Wqkv[1024:2048][r], Wqkv[2048:3072][r]], axis=0)
        in_maps.append({
            "xt": np.ascontiguousarray(x[b].T).astype(bf),
            "wt": np.ascontiguousarray(w_my.T).astype(bf),
            "wot": np.ascontiguousarray(Wout[:, r].T).astype(bf),
        })
    return in_maps


def gather_output(results):
    """8 per-core partial y (bf16) -> full [2, T, C] f32 output."""
    y = np.zeros((2, T, C), dtype=np.float64)
    for i in range(N_CORES):
        y[i // 4] += np.asarray(results[i]["y"], dtype=np.float64)
    return y.astype(np.float32)


def kernel(x, Wqkv, Wout):
    x = np.asarray(x)
    Wqkv = np.asarray(Wqkv)
    Wout = np.asarray(Wout)
    nc = build_program()
    in_maps = shard_inputs(x, Wqkv, Wout)
    res = run_bass_kernel_spmd(nc, in_maps, core_ids=list(range(N_CORES)))
    return gather_output(res.results)
